# revision 77
# baseline (speedup 1.0000x reference)
"""Trainium2 Bass kernel for single-head attention (no mask), fp8 DoubleRow.

Reference computation (B=4, S=2048, D=1024):
    q = x @ Wq.T ; k = x @ Wk.T ; v = x @ Wv.T          (per batch)
    out = softmax((q @ k.T) / sqrt(1024)) @ v

Sharding: 8 cores = (batch, query-half), same as the bf16 baseline; no
collectives.  Algebra: scores = x (Wq^T Wk) x^T with M = Wq^T Wk
host-prepped, out = softmax(scores) x Wv^T.

All four dense stages run as fp8e4 (e4m3) DoubleRow matmuls: the PE
contracts two 128-row k-tiles per instruction at 0.5 cycles/moving-row,
4x the bf16 MAC rate.  e4m3's ~3.6% quantization noise is managed by
hi+lo residual splitting (a = fp8(a) + fp8(a - fp8(a))) with the number
of product terms chosen per stage, and by a Taylor shift of the softmax:
    E = exp(s) = 1 + Etil,   C = E^T x = colsum(x) (+) Etil^T x
so the rank-1 mass of E (the dominant part) flows through an exact f32
side-channel (colsum via tiny ones-matmuls on the PE, ~free) and only
the small Etil (std ~0.37) is quantized -- cutting its error ~3x.

Stages (per core, q = the core's 1024 queries, 64x scale keeps fp8
operands out of the denormal range):
    A: H = (64 M)^T xq^T        [d,q]  terms m8*x8 + mr8*x8 + m8*r8
    B: S = x H                  [k,q]  terms x8*(h8 + hr8)
    Etil = exp(S/2048) [Act, f32] - 1 [DVE/Pool] -> fp8
    rowsum_q = 2048 + sum_k Etil   via ones-stationary PE chains
    C: Ct = Etil^T x            [d,q]  term  x8^T e8
    D: o = (64 Wv^T)^T C        [o,q]  terms (wv8+wvr8)*c8 + wv8*cr8
       + v0 = (64 Wv)^T colsum  [o,1]  via tiny [*,1] DoubleRow chains
    evict: out = (d_ps + v0/2) * (2/(64*rowsum)), one DVE STT -> bf16

Error (vs f32 reference, measured on device): 1.59e-2 mean-rel (gate
2e-2).  PE work: (49.2 + 65.5 + 32.8 + 49.2)k cycles ~= 82 us @ 2.4
GHz vs 393k cycles (164 us) for the bf16 baseline; cost-model total
94.9 us vs 172.3 us.

Scheduling: one in-order SP DMA queue ordered by first use, with
phase-A loads j-group-interleaved to match wave-0's j-major emission;
a warmup matmul chain bridges the initial DMA latency; every phase is
qc-major so each phase's qc-0 chains depend only on evictions that
finished half a phase earlier (no junction stalls, PE p-state holds);
tiny rowsum/colsum/v0 chains interleave into the C/D instruction
stream; the final output chain is split into two pipelined halves so
the tail is one narrow evict+store.
"""

import ml_dtypes
import numpy as np

import concourse.tile as tile
from concourse import bacc, mybir
from concourse.bass_utils import run_bass_kernel_spmd

B, S, D, O = 4, 2048, 1024, 1024
HQ = S // 2  # query rows per core
N_CORES = 8
BF = mybir.dt.bfloat16
F8 = mybir.dt.float8e4
F32 = mybir.dt.float32
DR = mybir.MatmulPerfMode.DoubleRow
EXP_SCALE = 1.0 / (32.0 * 64.0)  # softmax 1/sqrt(1024) and the 64x M scale
DP = D // 256  # 4 contraction pair-tiles over d
KP = S // 256  # 8 key pair-tiles
NWARM = 22  # warmup matmuls bridging the initial DMA latency

_CACHE: dict = {}


def _emit(nc, sfx=""):
    m8_d = nc.dram_tensor(f"m8{sfx}", [DP, 128, 2, D], F8, kind="ExternalInput")
    mr8_d = nc.dram_tensor(f"mr8{sfx}", [DP, 128, 2, D], F8, kind="ExternalInput")
    x8t_d = nc.dram_tensor(f"x8t{sfx}", [DP, 128, 2, S], F8, kind="ExternalInput")
    r8tq_d = nc.dram_tensor(f"r8tq{sfx}", [DP, 128, 2, HQ], F8, kind="ExternalInput")
    x8n_d = nc.dram_tensor(f"x8n{sfx}", [KP, 128, 2, D], F8, kind="ExternalInput")
    r8n_d = nc.dram_tensor(f"r8n{sfx}", [KP, 128, 2, D], F8, kind="ExternalInput")
    wv8_d = nc.dram_tensor(f"wv8{sfx}", [DP, 128, 2, O], F8, kind="ExternalInput")
    wvr8_d = nc.dram_tensor(f"wvr8{sfx}", [DP, 128, 2, O], F8, kind="ExternalInput")
    out_d = nc.dram_tensor(f"outT{sfx}", [O, HQ], BF, kind="ExternalOutput")

    with tile.TileContext(nc) as tc:
        with (
            tc.tile_pool(name=f"{sfx}sb", bufs=1) as sb,
            tc.tile_pool(name=f"{sfx}pp", bufs=7, space="PSUM") as pp,
            tc.tile_pool(name=f"{sfx}rs", bufs=1, space="PSUM") as rs,
        ):
            m8 = [sb.tile([128, 2, D], F8, tag=f"m8_{j}", name=f"m8{sfx}_{j}") for j in range(DP)]
            mr8 = [sb.tile([128, 2, D], F8, tag=f"mr8_{j}", name=f"mr8{sfx}_{j}") for j in range(DP)]
            x8t = [sb.tile([128, 2, S], F8, tag=f"x8t_{j}", name=f"x8t{sfx}_{j}") for j in range(DP)]
            r8tq = [sb.tile([128, 2, HQ], F8, tag=f"r8tq_{j}", name=f"r8tq{sfx}_{j}") for j in range(DP)]
            x8tq = [t[:, :, 0:HQ] for t in x8t]
            x8tk = [t[:, :, HQ:S] for t in x8t]
            x8n = [sb.tile([128, 2, D], F8, tag=f"x8n_{m}", name=f"x8n{sfx}_{m}") for m in range(KP)]
            r8n = [sb.tile([128, 2, D], F8, tag=f"r8n_{m}", name=f"r8n{sfx}_{m}") for m in range(KP)]
            wv8 = [sb.tile([128, 2, O], F8, tag=f"wv8_{j}", name=f"wv8{sfx}_{j}") for j in range(DP)]
            wvr8 = [sb.tile([128, 2, O], F8, tag=f"wvr8_{j}", name=f"wvr8{sfx}_{j}") for j in range(DP)]
            h8 = [sb.tile([128, 2, HQ], F8, tag=f"h8_{j}", name=f"h8{sfx}_{j}") for j in range(DP)]
            hr8 = [sb.tile([128, 2, HQ], F8, tag=f"hr8_{j}", name=f"hr8{sfx}_{j}") for j in range(DP)]
            et8 = [sb.tile([128, 2, HQ], F8, tag=f"et8_{m}", name=f"et8{sfx}_{m}") for m in range(KP)]
            c8 = [sb.tile([128, 2, HQ], F8, tag=f"c8_{j}", name=f"c8{sfx}_{j}") for j in range(DP)]
            cr8 = [sb.tile([128, 2, HQ], F8, tag=f"cr8_{j}", name=f"cr8{sfx}_{j}") for j in range(DP)]
            cs8 = sb.tile([128, 2, DP], F8, tag="cs8", name=f"cs8{sfx}")
            csr8 = sb.tile([128, 2, DP], F8, tag="csr8", name=f"csr8{sfx}")
            # 32 columns: a 2-column fp8 stationary trips walrus's
            # s3_lw_dual_fp8_restrictions ISA check in the rowsum chains.
            ones8 = sb.tile([128, 2, 32], F8, tag="ones8", name=f"ones8{sfx}")
            v0sb = sb.tile([128, 8], F32, tag="v0sb", name=f"v0sb{sfx}")
            warmt = sb.tile([128, 512], BF, tag="warm", name=f"warmt{sfx}")
            rsum = sb.tile([128, HQ], F32, tag="rsum", name=f"rsum{sfx}")
            recip2 = sb.tile([128, HQ], F32, tag="recip2", name=f"recip2{sfx}")

            # Constants via memset (no DMA bandwidth). warmt on DVE so the
            # warmup chain can start early; ones8 (fp8) on Pool.
            nc.gpsimd.memset(warmt, 0.0)
            nc.gpsimd.memset(ones8, 1.0)
            # Dummy exp: hoists the Act engine's one-time activation-table
            # load into the idle startup window.
            actwarm = sb.tile([128, 1], BF, tag="actwarm", name=f"actwarm{sfx}")
            nc.scalar.activation(
                out=actwarm,
                in_=warmt[:, 0:1],
                func=mybir.ActivationFunctionType.Exp,
                scale=EXP_SCALE,
            )

            # ---- DMA loads: one in-order queue = explicit priority ----
            # Phase A's inputs land j-group by j-group, matching wave-0's
            # j-major consumption; x8t is split so only the query columns are
            # on A's critical path (key columns follow before phase B).
            for j in range(DP):
                nc.sync.dma_start(out=m8[j], in_=m8_d[j])
                nc.sync.dma_start(out=x8tq[j], in_=x8t_d[j][:, :, 0:HQ])
                nc.sync.dma_start(out=mr8[j], in_=mr8_d[j])
                nc.sync.dma_start(out=r8tq[j], in_=r8tq_d[j])
            for j in range(DP):
                nc.sync.dma_start(out=x8tk[j], in_=x8t_d[j][:, :, HQ:S])
            for m in range(KP):
                nc.sync.dma_start(out=x8n[m], in_=x8n_d[m])
            for m in range(KP):
                nc.sync.dma_start(out=r8n[m], in_=r8n_d[m])
            for j in range(DP):
                nc.sync.dma_start(out=wv8[j], in_=wv8_d[j])
            for j in range(DP):
                nc.sync.dma_start(out=wvr8[j], in_=wvr8_d[j])

            # ---- PE warmup: hold the p-state until the first loads land ----
            wps = rs.tile([128, 512], F32, tag="rs", name=f"wps{sfx}")
            for i in range(NWARM):
                nc.tensor.matmul(
                    wps[:, 0:128], warmt[:, 0:128], warmt[:, 0:128],
                    start=True, stop=True,
                )

            # ---- Phase A: H = (64 M)^T xq^T, evicted as fp8 hi+lo ----
            # Wave 0 = qc 0 (so phase B's qc-0 chains can chase wave-0's
            # evictions), wave 1 = qc 1.  Terms j-major so the DMA feed
            # (m8, x8t first, then mr8, then r8tq) is consumed in order.
            for qc in range(2):
                a_ps = {}
                for d1t in range(8):
                    pool = pp if d1t < 7 else rs
                    tag = "ps" if d1t < 7 else "rs"
                    a_ps[d1t] = pool.tile([128, 512], F32, tag=tag, name=f"aps{sfx}_{qc}_{d1t}")
                qlo = qc * 512
                terms = [(m8, x8tq), (mr8, x8tq), (m8, r8tq)]
                # j-major: wave 0 consumes exactly one DMA j-group (m8[j],
                # x8t[j] queries, mr8[j], r8tq[j]) per j step, tracking the
                # in-order load queue instead of sprinting ahead of it.
                for j in range(DP - 1):
                    for lhs, rhs in terms:
                        for d1t in range(8):
                            nc.tensor.matmul(
                                a_ps[d1t],
                                lhs[j][:, :, d1t * 128 : (d1t + 1) * 128],
                                rhs[j][:, :, qlo : qlo + 512],
                                start=(j == 0 and lhs is m8 and rhs is x8tq),
                                stop=False,
                                perf_mode=DR,
                            )
                # Final j-group chain-major with inline evictions so chains
                # finish staggered and the eviction burst overlaps the wave.
                # d1t=7 (the rs-bank chain) evicts first: the next wave's
                # first loop needs that single-buffered bank back.
                for d1t in [7, 0, 1, 2, 3, 4, 5, 6]:
                    for t, (lhs, rhs) in enumerate(terms):
                        nc.tensor.matmul(
                            a_ps[d1t],
                            lhs[DP - 1][:, :, d1t * 128 : (d1t + 1) * 128],
                            rhs[DP - 1][:, :, qlo : qlo + 512],
                            start=False,
                            stop=(t == 2),
                            perf_mode=DR,
                        )
                    dst8 = h8[d1t // 2][:, d1t % 2, qc * 512 : (qc + 1) * 512]
                    dstr = hr8[d1t // 2][:, d1t % 2, qc * 512 : (qc + 1) * 512]
                    nc.scalar.activation(
                        out=dst8,
                        in_=a_ps[d1t],
                        func=mybir.ActivationFunctionType.Copy,
                    )
                    nc.vector.scalar_tensor_tensor(
                        out=dstr,
                        in0=a_ps[d1t],
                        scalar=0.0,
                        in1=dst8,
                        op0=mybir.AluOpType.bypass,
                        op1=mybir.AluOpType.subtract,
                    )

            # ---- Phase B: S = x H; Etil = exp(S/2048) - 1 -> fp8; rowsums --
            for qc in range(2):
                for kt in range(16):
                    sp = pp.tile([128, 512], F32, tag="ps", name=f"sps{sfx}_{kt}_{qc}")
                    xsrc, ki = (x8tq, kt) if kt < 8 else (x8tk, kt - 8)
                    for term in range(2):
                        hsrc = h8 if term == 0 else hr8
                        for j in range(DP):
                            nc.tensor.matmul(
                                sp,
                                xsrc[j][:, :, ki * 128 : (ki + 1) * 128],
                                hsrc[j][:, :, qc * 512 : (qc + 1) * 512],
                                start=(term == 0 and j == 0),
                                stop=(term == 1 and j == DP - 1),
                                perf_mode=DR,
                            )
                    # E = exp(s) in f32 (Act), then Etil = E - 1 -> fp8 (Pool);
                    # rowsum reduces the exact f32 E (Pool) into rowacc (DVE).
                    ef32 = sb.tile(
                        [128, 512], F32, tag=f"ef32{qc}", bufs=3,
                        name=f"ef32{sfx}_{kt}_{qc}",
                    )
                    nc.scalar.activation(
                        out=ef32,
                        in_=sp,
                        func=mybir.ActivationFunctionType.Exp,
                        scale=EXP_SCALE,
                    )
                    # Etil = E - 1 -> fp8 (DVE/Pool alternate; Act only does
                    # the exp, so no engine saturates the 853ns chain period).
                    edst = et8[kt // 2][:, kt % 2, qc * 512 : (qc + 1) * 512]
                    if kt % 2 == 0:
                        nc.vector.tensor_scalar_sub(edst, ef32, 1.0)
                    else:
                        nc.gpsimd.tensor_scalar_sub(edst, ef32, 1.0)

            # ---- Phase C: Ct = Etil^T x (1 term), colsum chains woven in ---
            # colsum chain dt: [128,1] psum over all 2048 keys of x8n + r8n.
            csp = {}

            def colsum_chain(dt):
                csp[dt] = rs.tile([128, 512], F32, tag="rs", name=f"csp{sfx}_{dt}")[:, 0:1]
                for src in (x8n, r8n):
                    for m in range(KP):
                        nc.tensor.matmul(
                            csp[dt],
                            src[m][:, :, dt * 128 : (dt + 1) * 128],
                            ones8[:, :, 0:1],
                            start=(src is x8n and m == 0),
                            stop=(src is r8n and m == KP - 1),
                            perf_mode=DR,
                        )

            def colsum_evict(dt):
                c_hi = cs8[:, dt % 2, dt // 2 : dt // 2 + 1]
                nc.scalar.activation(
                    out=c_hi, in_=csp[dt],
                    func=mybir.ActivationFunctionType.Copy, scale=0.125,
                )
                nc.vector.scalar_tensor_tensor(
                    out=csr8[:, dt % 2, dt // 2 : dt // 2 + 1],
                    in0=csp[dt], scalar=0.125, in1=c_hi,
                    op0=mybir.AluOpType.mult,
                    op1=mybir.AluOpType.subtract,
                )

            # Softmax denominators on the PE: rowsum_q = sum_k Etil[k,q] via
            # ones-stationary chains ([32,512] psum each), then
            # recip2 = 2 / (64 * (2048 + rowsum)) broadcast to all partitions
            # (the 2 un-scales c8 = C/2, the 64 un-scales wv = 64 Wv^T).
            def rowsum_chain(qc):
                r_ps = pp.tile([128, 512], F32, tag="ps", name=f"rps{sfx}_{qc}")[0:32, :]
                for m in range(KP):
                    nc.tensor.matmul(
                        r_ps,
                        ones8,
                        et8[m][:, :, qc * 512 : (qc + 1) * 512],
                        start=(m == 0),
                        stop=(m == KP - 1),
                        perf_mode=DR,
                    )
                nc.vector.tensor_scalar_add(
                    rsum[0:1, qc * 512 : (qc + 1) * 512], r_ps[0:1, :], 2048.0
                )

            def c_chain(dt, qc):
                c_ps = pp.tile([128, 512], F32, tag="ps", name=f"cps{sfx}_{dt}_{qc}")
                for m in range(KP):
                    nc.tensor.matmul(
                        c_ps,
                        x8n[m][:, :, dt * 128 : (dt + 1) * 128],
                        et8[m][:, :, qc * 512 : (qc + 1) * 512],
                        start=(m == 0),
                        stop=(m == KP - 1),
                        perf_mode=DR,
                    )
                cdst8 = c8[dt // 2][:, dt % 2, qc * 512 : (qc + 1) * 512]
                nc.scalar.activation(
                    out=cdst8,
                    in_=c_ps,
                    func=mybir.ActivationFunctionType.Copy,
                    scale=0.5,
                )
                nc.vector.scalar_tensor_tensor(
                    out=cr8[dt // 2][:, dt % 2, qc * 512 : (qc + 1) * 512],
                    in0=c_ps, scalar=0.5, in1=cdst8,
                    op0=mybir.AluOpType.mult,
                    op1=mybir.AluOpType.subtract,
                )

            # qc-major: the qc-0 chains depend only on qc-0 evictions, which
            # finished half of phase B ago, so there is no junction stall.
            rowsum_chain(0)
            for dt in range(8):
                c_chain(dt, 0)
                if dt % 2 == 1:
                    colsum_chain(dt // 2)
                    colsum_evict(dt // 2)
            rowsum_chain(1)
            rb = rsum[0:1, :]
            nc.vector.reciprocal(out=rb, in_=rb)
            nc.vector.tensor_scalar_mul(rb, rb, 1.0 / 32.0)
            nc.gpsimd.partition_broadcast(recip2, rb, 128)
            for dt in range(8):
                c_chain(dt, 1)
                if dt % 2 == 0:
                    colsum_chain(4 + dt // 2)
                    colsum_evict(4 + dt // 2)

            # ---- Phase D: o = (64 Wv)^T C + v0, normalized at eviction ----
            # v0 chain ot: [128,1] psum = (64 Wv)^T (colsum/8); v0sb = x4
            # so the D-evict STT sees v0/2 on the same scale as d_ps = o64/2.
            def v0_chain(ot):
                vp = rs.tile([128, 512], F32, tag="rs", name=f"vp{sfx}_{ot}")[:, 0:1]
                terms = [(wv8, cs8), (wvr8, cs8), (wv8, csr8)]
                for t, (wsrc, csrc) in enumerate(terms):
                    for j in range(DP):
                        nc.tensor.matmul(
                            vp,
                            wsrc[j][:, :, ot * 128 : (ot + 1) * 128],
                            csrc[:, :, j : j + 1],
                            start=(t == 0 and j == 0),
                            stop=(t == 2 and j == DP - 1),
                            perf_mode=DR,
                        )
                nc.vector.tensor_scalar_mul(v0sb[:, ot : ot + 1], vp, 4.0)

            v0_chain(0)
            v0_chain(1)

            def d_chain(ot, qc, col0, ncol):
                d_ps = pp.tile(
                    [128, 512], F32, tag="ps", name=f"dps{sfx}_{ot}_{col0}"
                )[:, 0:ncol]
                terms = [(wv8, c8), (wvr8, c8), (wv8, cr8)]
                for t, (wsrc, csrc) in enumerate(terms):
                    for j in range(DP):
                        nc.tensor.matmul(
                            d_ps,
                            wsrc[j][:, :, ot * 128 : (ot + 1) * 128],
                            csrc[j][:, :, col0 : col0 + ncol],
                            start=(t == 0 and j == 0),
                            stop=(t == 2 and j == DP - 1),
                            perf_mode=DR,
                        )
                oev = sb.tile(
                    [128, ncol], BF, tag=f"oev{ncol}", bufs=4,
                    name=f"oev{sfx}_{ot}_{col0}",
                )
                nc.vector.scalar_tensor_tensor(
                    out=oev,
                    in0=d_ps,
                    scalar=v0sb[:, ot : ot + 1],
                    in1=recip2[:, col0 : col0 + ncol],
                    op0=mybir.AluOpType.add,
                    op1=mybir.AluOpType.mult,
                )
                # SP queue only: gpsimd DMA takes the slow SWDGE path (~1us
                # Pool-side descriptor gen) which bloats the tail.
                nc.sync.dma_start(
                    out=out_d[ot * 128 : (ot + 1) * 128, col0 : col0 + ncol],
                    in_=oev,
                )

            # qc-major: D's qc-0 chains only need C's qc-0 evictions.
            for ch in range(15):
                qc, ot = divmod(ch, 8)
                d_chain(ot, qc, qc * 512, 512)
                if 0 <= ch < 6:
                    v0_chain(ch + 2)
            # Final chain as two pipelined halves: the first half's eviction
            # and store descriptor-gen overlap the second half's matmuls.
            d_chain(7, 1, 512, 256)
            d_chain(7, 1, 768, 256)
    return nc


def _get_program():
    if "nc" not in _CACHE:
        nc = bacc.Bacc("TRN2", target_bir_lowering=False, num_devices=N_CORES)
        _emit(nc)
        nc.compile()
        _CACHE["nc"] = nc
    return _CACHE["nc"]


def _split8(a):
    f8 = ml_dtypes.float8_e4m3
    hi = a.astype(f8)
    lo = (a - hi.astype(np.float32)).astype(f8)
    return hi, lo


def _pair_t(a):
    """[R, C] -> [R/256, 128, 2, C]: partition-dim tile pairs for DoubleRow."""
    r, c = a.shape
    return np.ascontiguousarray(a.reshape(r // 256, 2, 128, c).transpose(0, 2, 1, 3))


def kernel(x, Wq, Wk, Wv):
    x = np.asarray(x, dtype=np.float32)
    Wq = np.asarray(Wq, dtype=np.float32)
    Wk = np.asarray(Wk, dtype=np.float32)
    Wv = np.asarray(Wv, dtype=np.float32)

    nc = _get_program()
    m8, mr8 = _split8(64.0 * (Wq.T @ Wk))       # [d2, d1], 64x scale
    wv8, wvr8 = _split8(64.0 * Wv.T)            # [d, o], 64x scale
    m8p, mr8p = _pair_t(m8), _pair_t(mr8)
    wv8p, wvr8p = _pair_t(wv8), _pair_t(wvr8)
    in_maps = []
    for c in range(N_CORES):
        b, h = divmod(c, 2)
        xp = np.concatenate(
            [x[b, h * HQ : (h + 1) * HQ], x[b, (1 - h) * HQ : (2 - h) * HQ]], axis=0
        )
        x8, r8 = _split8(xp)                    # [k, d]
        x8t = np.ascontiguousarray(x8.astype(np.float32).T).astype(ml_dtypes.float8_e4m3)
        r8t = np.ascontiguousarray(r8.astype(np.float32).T).astype(ml_dtypes.float8_e4m3)
        in_maps.append(
            {
                "m8": m8p, "mr8": mr8p,
                "x8t": _pair_t(x8t),
                "r8tq": _pair_t(r8t[:, 0:HQ]),
                "x8n": _pair_t(x8), "r8n": _pair_t(r8),
                "wv8": wv8p, "wvr8": wvr8p,
            }
        )
    res = None
    for attempt in range(3):
        try:
            res = run_bass_kernel_spmd(nc, in_maps, list(range(N_CORES)))
            break
        except Exception:
            # The axon PJRT path sporadically fails with
            # NRT_EXEC_UNIT_UNRECOVERABLE on an otherwise-good program;
            # a retry on a fresh execute has always succeeded.
            if attempt == 2:
                raise
    assert res is not None
    outp = np.empty((B, S, O), dtype=np.float32)
    for c in range(N_CORES):
        b, h = divmod(c, 2)
        outp[b, h * HQ : (h + 1) * HQ] = res.results[c]["outT"].astype(np.float32).T
    return outp


# revision 78
# speedup vs baseline: 1.0374x; 1.0374x over previous
"""Trainium2 Bass kernel for single-head attention (no mask), fp8 DoubleRow.

Reference computation (B=4, S=2048, D=1024):
    q = x @ Wq.T ; k = x @ Wk.T ; v = x @ Wv.T          (per batch)
    out = softmax((q @ k.T) / sqrt(1024)) @ v

Sharding: 8 cores = (batch, query-half), same as the bf16 baseline; no
collectives.  Algebra: scores = x (Wq^T Wk) x^T with M = Wq^T Wk
host-prepped, out = softmax(scores) x Wv^T.

All four dense stages run as fp8e4 (e4m3) DoubleRow matmuls: the PE
contracts two 128-row k-tiles per instruction at 0.5 cycles/moving-row,
4x the bf16 MAC rate.  e4m3's ~3.6% quantization noise is managed by
hi+lo residual splitting (a = fp8(a) + fp8(a - fp8(a))) with the number
of product terms chosen per stage, and by a Taylor shift of the softmax:
    E = exp(s) = 1 + Etil,   C = E^T x = colsum(x) (+) Etil^T x
so the rank-1 mass of E (the dominant part) flows through an exact f32
side-channel (colsum via tiny ones-matmuls on the PE, ~free) and only
the small Etil (std ~0.37) is quantized -- cutting its error ~3x.

Stages (per core, q = the core's 1024 queries, 64x scale keeps fp8
operands out of the denormal range):
    A: H = (64 M)^T xq^T        [d,q]  terms m8*x8 + mr8*x8 + m8*r8
    B: S = x H                  [k,q]  terms x8*(h8 + hr8)
    Etil = exp(S/2048) [Act, f32] - 1 [DVE/Pool] -> fp8
    rowsum_q = 2048 + sum_k Etil   via ones-stationary PE chains
    C: Ct = Etil^T x            [d,q]  term  x8^T e8
    D: o = (64 Wv^T)^T C        [o,q]  terms (wv8+wvr8)*c8 + wv8*cr8
       + v0 = (64 Wv)^T colsum  [o,1]  via tiny [*,1] DoubleRow chains
    evict: out = (d_ps + v0/2) * (2/(64*rowsum)), one DVE STT -> bf16

Error (vs f32 reference, measured on device): 1.59e-2 mean-rel (gate
2e-2).  PE work: (49.2 + 65.5 + 32.8 + 49.2)k cycles ~= 82 us @ 2.4
GHz vs 393k cycles (164 us) for the bf16 baseline; cost-model total
94.9 us vs 172.3 us.

Scheduling: one in-order SP DMA queue ordered by first use, with
phase-A loads j-group-interleaved to match wave-0's j-major emission;
a warmup matmul chain bridges the initial DMA latency; every phase is
qc-major so each phase's qc-0 chains depend only on evictions that
finished half a phase earlier (no junction stalls, PE p-state holds);
tiny rowsum/colsum/v0 chains interleave into the C/D instruction
stream; the final output chain is split into two pipelined halves so
the tail is one narrow evict+store.
"""

import ml_dtypes
import numpy as np

import concourse.tile as tile
from concourse import bacc, mybir
from concourse.bass_utils import run_bass_kernel_spmd

B, S, D, O = 4, 2048, 1024, 1024
HQ = S // 2  # query rows per core
N_CORES = 8
BF = mybir.dt.bfloat16
F8 = mybir.dt.float8e4
F32 = mybir.dt.float32
DR = mybir.MatmulPerfMode.DoubleRow
EXP_SCALE = 1.0 / (32.0 * 64.0)  # softmax 1/sqrt(1024) and the 64x M scale
DP = D // 256  # 4 contraction pair-tiles over d
KP = S // 256  # 8 key pair-tiles
NWARM = 22  # warmup matmuls bridging the initial DMA latency

_CACHE: dict = {}


def _emit(nc, sfx=""):
    m8_d = nc.dram_tensor(f"m8{sfx}", [DP, 128, 2, D], F8, kind="ExternalInput")
    mr8_d = nc.dram_tensor(f"mr8{sfx}", [DP, 128, 2, D], F8, kind="ExternalInput")
    x8t_d = nc.dram_tensor(f"x8t{sfx}", [DP, 128, 2, S], F8, kind="ExternalInput")
    r8tq_d = nc.dram_tensor(f"r8tq{sfx}", [DP, 128, 2, HQ], F8, kind="ExternalInput")
    x8n_d = nc.dram_tensor(f"x8n{sfx}", [KP, 128, 2, D], F8, kind="ExternalInput")
    r8n_d = nc.dram_tensor(f"r8n{sfx}", [KP, 128, 2, D], F8, kind="ExternalInput")
    wv8_d = nc.dram_tensor(f"wv8{sfx}", [DP, 128, 2, O], F8, kind="ExternalInput")
    wvr8_d = nc.dram_tensor(f"wvr8{sfx}", [DP, 128, 2, O], F8, kind="ExternalInput")
    out_d = nc.dram_tensor(f"outT{sfx}", [O, HQ], BF, kind="ExternalOutput")

    with tile.TileContext(nc) as tc:
        with (
            tc.tile_pool(name=f"{sfx}sb", bufs=1) as sb,
            tc.tile_pool(name=f"{sfx}pp", bufs=7, space="PSUM") as pp,
            tc.tile_pool(name=f"{sfx}rs", bufs=1, space="PSUM") as rs,
        ):
            m8 = [sb.tile([128, 2, D], F8, tag=f"m8_{j}", name=f"m8{sfx}_{j}") for j in range(DP)]
            mr8 = [sb.tile([128, 2, D], F8, tag=f"mr8_{j}", name=f"mr8{sfx}_{j}") for j in range(DP)]
            x8t = [sb.tile([128, 2, S], F8, tag=f"x8t_{j}", name=f"x8t{sfx}_{j}") for j in range(DP)]
            r8tq = [sb.tile([128, 2, HQ], F8, tag=f"r8tq_{j}", name=f"r8tq{sfx}_{j}") for j in range(DP)]
            x8tq = [t[:, :, 0:HQ] for t in x8t]
            x8tk = [t[:, :, HQ:S] for t in x8t]
            x8n = [sb.tile([128, 2, D], F8, tag=f"x8n_{m}", name=f"x8n{sfx}_{m}") for m in range(KP)]
            r8n = [sb.tile([128, 2, D], F8, tag=f"r8n_{m}", name=f"r8n{sfx}_{m}") for m in range(KP)]
            wv8 = [sb.tile([128, 2, O], F8, tag=f"wv8_{j}", name=f"wv8{sfx}_{j}") for j in range(DP)]
            wvr8 = [sb.tile([128, 2, O], F8, tag=f"wvr8_{j}", name=f"wvr8{sfx}_{j}") for j in range(DP)]
            h8 = [sb.tile([128, 2, HQ], F8, tag=f"h8_{j}", name=f"h8{sfx}_{j}") for j in range(DP)]
            hr8 = [sb.tile([128, 2, HQ], F8, tag=f"hr8_{j}", name=f"hr8{sfx}_{j}") for j in range(DP)]
            et8 = [sb.tile([128, 2, HQ], F8, tag=f"et8_{m}", name=f"et8{sfx}_{m}") for m in range(KP)]
            c8 = [sb.tile([128, 2, HQ], F8, tag=f"c8_{j}", name=f"c8{sfx}_{j}") for j in range(DP)]
            cr8 = [sb.tile([128, 2, HQ], F8, tag=f"cr8_{j}", name=f"cr8{sfx}_{j}") for j in range(DP)]
            cs8 = sb.tile([128, 2, DP], F8, tag="cs8", name=f"cs8{sfx}")
            csr8 = sb.tile([128, 2, DP], F8, tag="csr8", name=f"csr8{sfx}")
            # 32 columns: a 2-column fp8 stationary trips walrus's
            # s3_lw_dual_fp8_restrictions ISA check in the rowsum chains.
            ones8 = sb.tile([128, 2, 32], F8, tag="ones8", name=f"ones8{sfx}")
            v0sb = sb.tile([128, 8], F32, tag="v0sb", name=f"v0sb{sfx}")
            warmt = sb.tile([128, 512], BF, tag="warm", name=f"warmt{sfx}")
            rsum = sb.tile([128, HQ], F32, tag="rsum", name=f"rsum{sfx}")
            recip2 = sb.tile([128, HQ], F32, tag="recip2", name=f"recip2{sfx}")

            # Constants via memset (no DMA bandwidth). warmt on DVE so the
            # warmup chain can start early; ones8 (fp8) on Pool.
            nc.gpsimd.memset(warmt, 0.0)
            nc.gpsimd.memset(ones8, 1.0)
            # Dummy exp: hoists the Act engine's one-time activation-table
            # load into the idle startup window.
            actwarm = sb.tile([128, 1], BF, tag="actwarm", name=f"actwarm{sfx}")
            nc.scalar.activation(
                out=actwarm,
                in_=warmt[:, 0:1],
                func=mybir.ActivationFunctionType.Exp,
                scale=EXP_SCALE,
            )

            # ---- DMA loads: one in-order queue = explicit priority ----
            # Phase A's inputs land j-group by j-group, matching wave-0's
            # j-major consumption; x8t is split so only the query columns are
            # on A's critical path (key columns follow before phase B).
            for j in range(DP):
                nc.sync.dma_start(out=m8[j], in_=m8_d[j])
                nc.sync.dma_start(out=x8tq[j], in_=x8t_d[j][:, :, 0:HQ])
                nc.sync.dma_start(out=mr8[j], in_=mr8_d[j])
                nc.sync.dma_start(out=r8tq[j], in_=r8tq_d[j])
            for j in range(DP):
                nc.sync.dma_start(out=x8tk[j], in_=x8t_d[j][:, :, HQ:S])
            for m in range(KP):
                nc.sync.dma_start(out=x8n[m], in_=x8n_d[m])
            for m in range(KP):
                nc.sync.dma_start(out=r8n[m], in_=r8n_d[m])
            for j in range(DP):
                nc.sync.dma_start(out=wv8[j], in_=wv8_d[j])
            for j in range(DP):
                nc.sync.dma_start(out=wvr8[j], in_=wvr8_d[j])

            # ---- PE warmup: hold the p-state until the first loads land ----
            wps = rs.tile([128, 512], F32, tag="rs", name=f"wps{sfx}")
            for i in range(NWARM):
                nc.tensor.matmul(
                    wps[:, 0:128], warmt[:, 0:128], warmt[:, 0:128],
                    start=True, stop=True,
                )

            # ---- Phase A: H = (64 M)^T xq^T, evicted as fp8 hi+lo ----
            # Wave 0 = qc 0 (so phase B's qc-0 chains can chase wave-0's
            # evictions), wave 1 = qc 1.  Terms j-major so the DMA feed
            # (m8, x8t first, then mr8, then r8tq) is consumed in order.
            for qc in range(2):
                a_ps = {}
                for d1t in range(8):
                    pool = pp if d1t < 7 else rs
                    tag = "ps" if d1t < 7 else "rs"
                    a_ps[d1t] = pool.tile([128, 512], F32, tag=tag, name=f"aps{sfx}_{qc}_{d1t}")
                qlo = qc * 512
                terms = [(m8, x8tq), (mr8, x8tq), (m8, r8tq)]
                # j-major: wave 0 consumes exactly one DMA j-group (m8[j],
                # x8t[j] queries, mr8[j], r8tq[j]) per j step, tracking the
                # in-order load queue instead of sprinting ahead of it.
                for j in range(DP - 1):
                    for lhs, rhs in terms:
                        for d1t in range(8):
                            nc.tensor.matmul(
                                a_ps[d1t],
                                lhs[j][:, :, d1t * 128 : (d1t + 1) * 128],
                                rhs[j][:, :, qlo : qlo + 512],
                                start=(j == 0 and lhs is m8 and rhs is x8tq),
                                stop=False,
                                perf_mode=DR,
                            )
                # Final j-group chain-major with inline evictions so chains
                # finish staggered and the eviction burst overlaps the wave.
                # d1t=7 (the rs-bank chain) evicts first: the next wave's
                # first loop needs that single-buffered bank back.
                for d1t in [7, 0, 1, 2, 3, 4, 5, 6]:
                    for t, (lhs, rhs) in enumerate(terms):
                        nc.tensor.matmul(
                            a_ps[d1t],
                            lhs[DP - 1][:, :, d1t * 128 : (d1t + 1) * 128],
                            rhs[DP - 1][:, :, qlo : qlo + 512],
                            start=False,
                            stop=(t == 2),
                            perf_mode=DR,
                        )
                    dst8 = h8[d1t // 2][:, d1t % 2, qc * 512 : (qc + 1) * 512]
                    dstr = hr8[d1t // 2][:, d1t % 2, qc * 512 : (qc + 1) * 512]
                    nc.scalar.activation(
                        out=dst8,
                        in_=a_ps[d1t],
                        func=mybir.ActivationFunctionType.Copy,
                    )
                    nc.vector.scalar_tensor_tensor(
                        out=dstr,
                        in0=a_ps[d1t],
                        scalar=0.0,
                        in1=dst8,
                        op0=mybir.AluOpType.bypass,
                        op1=mybir.AluOpType.subtract,
                    )

            # ---- Phase B: S = x H; Etil = exp(S/2048) - 1 -> fp8; rowsums --
            for qc in range(2):
                for kt in range(16):
                    sp = pp.tile([128, 512], F32, tag="ps", name=f"sps{sfx}_{kt}_{qc}")
                    # The hr8 (H-requant residual) term covers d-dims 0..767
                    # only: the last quarter's correction is skipped, trading
                    # ~0.8e-2 of (quadrature) error for 8.2k PE cycles.
                    # Measured total: 1.78e-2 vs the 2e-2 gate.
                    xsrc, ki = (x8tq, kt) if kt < 8 else (x8tk, kt - 8)
                    for term, hsrc, nj in ((0, h8, DP), (1, hr8, DP - 1)):
                        for j in range(nj):
                            nc.tensor.matmul(
                                sp,
                                xsrc[j][:, :, ki * 128 : (ki + 1) * 128],
                                hsrc[j][:, :, qc * 512 : (qc + 1) * 512],
                                start=(term == 0 and j == 0),
                                stop=(term == 1 and j == nj - 1),
                                perf_mode=DR,
                            )
                    # E = exp(s) in f32 (Act), then Etil = E - 1 -> fp8 (Pool);
                    # rowsum reduces the exact f32 E (Pool) into rowacc (DVE).
                    ef32 = sb.tile(
                        [128, 512], F32, tag=f"ef32{qc}", bufs=3,
                        name=f"ef32{sfx}_{kt}_{qc}",
                    )
                    nc.scalar.activation(
                        out=ef32,
                        in_=sp,
                        func=mybir.ActivationFunctionType.Exp,
                        scale=EXP_SCALE,
                    )
                    # Etil = E - 1 -> fp8 (DVE/Pool alternate; Act only does
                    # the exp, so no engine saturates the 853ns chain period).
                    edst = et8[kt // 2][:, kt % 2, qc * 512 : (qc + 1) * 512]
                    if kt % 2 == 0:
                        nc.vector.tensor_scalar_sub(edst, ef32, 1.0)
                    else:
                        nc.gpsimd.tensor_scalar_sub(edst, ef32, 1.0)

            # ---- Phase C: Ct = Etil^T x (1 term), colsum chains woven in ---
            # colsum chain dt: [128,1] psum over all 2048 keys of x8n + r8n.
            csp = {}

            def colsum_chain(dt):
                csp[dt] = rs.tile([128, 512], F32, tag="rs", name=f"csp{sfx}_{dt}")[:, 0:1]
                for src in (x8n, r8n):
                    for m in range(KP):
                        nc.tensor.matmul(
                            csp[dt],
                            src[m][:, :, dt * 128 : (dt + 1) * 128],
                            ones8[:, :, 0:1],
                            start=(src is x8n and m == 0),
                            stop=(src is r8n and m == KP - 1),
                            perf_mode=DR,
                        )

            def colsum_evict(dt):
                c_hi = cs8[:, dt % 2, dt // 2 : dt // 2 + 1]
                nc.scalar.activation(
                    out=c_hi, in_=csp[dt],
                    func=mybir.ActivationFunctionType.Copy, scale=0.125,
                )
                nc.vector.scalar_tensor_tensor(
                    out=csr8[:, dt % 2, dt // 2 : dt // 2 + 1],
                    in0=csp[dt], scalar=0.125, in1=c_hi,
                    op0=mybir.AluOpType.mult,
                    op1=mybir.AluOpType.subtract,
                )

            # Softmax denominators on the PE: rowsum_q = sum_k Etil[k,q] via
            # ones-stationary chains ([32,512] psum each), then
            # recip2 = 2 / (64 * (2048 + rowsum)) broadcast to all partitions
            # (the 2 un-scales c8 = C/2, the 64 un-scales wv = 64 Wv^T).
            def rowsum_chain(qc):
                r_ps = pp.tile([128, 512], F32, tag="ps", name=f"rps{sfx}_{qc}")[0:32, :]
                for m in range(KP):
                    nc.tensor.matmul(
                        r_ps,
                        ones8,
                        et8[m][:, :, qc * 512 : (qc + 1) * 512],
                        start=(m == 0),
                        stop=(m == KP - 1),
                        perf_mode=DR,
                    )
                nc.vector.tensor_scalar_add(
                    rsum[0:1, qc * 512 : (qc + 1) * 512], r_ps[0:1, :], 2048.0
                )

            def c_chain(dt, qc):
                c_ps = pp.tile([128, 512], F32, tag="ps", name=f"cps{sfx}_{dt}_{qc}")
                for m in range(KP):
                    nc.tensor.matmul(
                        c_ps,
                        x8n[m][:, :, dt * 128 : (dt + 1) * 128],
                        et8[m][:, :, qc * 512 : (qc + 1) * 512],
                        start=(m == 0),
                        stop=(m == KP - 1),
                        perf_mode=DR,
                    )
                cdst8 = c8[dt // 2][:, dt % 2, qc * 512 : (qc + 1) * 512]
                nc.scalar.activation(
                    out=cdst8,
                    in_=c_ps,
                    func=mybir.ActivationFunctionType.Copy,
                    scale=0.5,
                )
                nc.vector.scalar_tensor_tensor(
                    out=cr8[dt // 2][:, dt % 2, qc * 512 : (qc + 1) * 512],
                    in0=c_ps, scalar=0.5, in1=cdst8,
                    op0=mybir.AluOpType.mult,
                    op1=mybir.AluOpType.subtract,
                )

            # qc-major: the qc-0 chains depend only on qc-0 evictions, which
            # finished half of phase B ago, so there is no junction stall.
            rowsum_chain(0)
            for dt in range(8):
                c_chain(dt, 0)
                if dt % 2 == 1:
                    colsum_chain(dt // 2)
                    colsum_evict(dt // 2)
            rowsum_chain(1)
            rb = rsum[0:1, :]
            nc.vector.reciprocal(out=rb, in_=rb)
            nc.vector.tensor_scalar_mul(rb, rb, 1.0 / 32.0)
            nc.gpsimd.partition_broadcast(recip2, rb, 128)
            for dt in range(8):
                c_chain(dt, 1)
                if dt % 2 == 0:
                    colsum_chain(4 + dt // 2)
                    colsum_evict(4 + dt // 2)

            # ---- Phase D: o = (64 Wv)^T C + v0, normalized at eviction ----
            # v0 chain ot: [128,1] psum = (64 Wv)^T (colsum/8); v0sb = x4
            # so the D-evict STT sees v0/2 on the same scale as d_ps = o64/2.
            def v0_chain(ot):
                vp = rs.tile([128, 512], F32, tag="rs", name=f"vp{sfx}_{ot}")[:, 0:1]
                terms = [(wv8, cs8), (wvr8, cs8), (wv8, csr8)]
                for t, (wsrc, csrc) in enumerate(terms):
                    for j in range(DP):
                        nc.tensor.matmul(
                            vp,
                            wsrc[j][:, :, ot * 128 : (ot + 1) * 128],
                            csrc[:, :, j : j + 1],
                            start=(t == 0 and j == 0),
                            stop=(t == 2 and j == DP - 1),
                            perf_mode=DR,
                        )
                nc.vector.tensor_scalar_mul(v0sb[:, ot : ot + 1], vp, 4.0)

            v0_chain(0)
            v0_chain(1)

            def d_chain(ot, qc, col0, ncol):
                d_ps = pp.tile(
                    [128, 512], F32, tag="ps", name=f"dps{sfx}_{ot}_{col0}"
                )[:, 0:ncol]
                terms = [(wv8, c8), (wvr8, c8), (wv8, cr8)]
                for t, (wsrc, csrc) in enumerate(terms):
                    for j in range(DP):
                        nc.tensor.matmul(
                            d_ps,
                            wsrc[j][:, :, ot * 128 : (ot + 1) * 128],
                            csrc[j][:, :, col0 : col0 + ncol],
                            start=(t == 0 and j == 0),
                            stop=(t == 2 and j == DP - 1),
                            perf_mode=DR,
                        )
                oev = sb.tile(
                    [128, ncol], BF, tag=f"oev{ncol}", bufs=4,
                    name=f"oev{sfx}_{ot}_{col0}",
                )
                nc.vector.scalar_tensor_tensor(
                    out=oev,
                    in0=d_ps,
                    scalar=v0sb[:, ot : ot + 1],
                    in1=recip2[:, col0 : col0 + ncol],
                    op0=mybir.AluOpType.add,
                    op1=mybir.AluOpType.mult,
                )
                # SP queue only: gpsimd DMA takes the slow SWDGE path (~1us
                # Pool-side descriptor gen) which bloats the tail.
                nc.sync.dma_start(
                    out=out_d[ot * 128 : (ot + 1) * 128, col0 : col0 + ncol],
                    in_=oev,
                )

            # qc-major: D's qc-0 chains only need C's qc-0 evictions.
            for ch in range(15):
                qc, ot = divmod(ch, 8)
                d_chain(ot, qc, qc * 512, 512)
                if 0 <= ch < 6:
                    v0_chain(ch + 2)
            # Final chain as two pipelined halves: the first half's eviction
            # and store descriptor-gen overlap the second half's matmuls.
            d_chain(7, 1, 512, 256)
            d_chain(7, 1, 768, 256)
    return nc


def _get_program():
    if "nc" not in _CACHE:
        nc = bacc.Bacc("TRN2", target_bir_lowering=False, num_devices=N_CORES)
        _emit(nc)
        nc.compile()
        _CACHE["nc"] = nc
    return _CACHE["nc"]


def _split8(a):
    f8 = ml_dtypes.float8_e4m3
    hi = a.astype(f8)
    lo = (a - hi.astype(np.float32)).astype(f8)
    return hi, lo


def _pair_t(a):
    """[R, C] -> [R/256, 128, 2, C]: partition-dim tile pairs for DoubleRow."""
    r, c = a.shape
    return np.ascontiguousarray(a.reshape(r // 256, 2, 128, c).transpose(0, 2, 1, 3))


def kernel(x, Wq, Wk, Wv):
    x = np.asarray(x, dtype=np.float32)
    Wq = np.asarray(Wq, dtype=np.float32)
    Wk = np.asarray(Wk, dtype=np.float32)
    Wv = np.asarray(Wv, dtype=np.float32)

    nc = _get_program()
    m8, mr8 = _split8(64.0 * (Wq.T @ Wk))       # [d2, d1], 64x scale
    wv8, wvr8 = _split8(64.0 * Wv.T)            # [d, o], 64x scale
    m8p, mr8p = _pair_t(m8), _pair_t(mr8)
    wv8p, wvr8p = _pair_t(wv8), _pair_t(wvr8)
    in_maps = []
    for c in range(N_CORES):
        b, h = divmod(c, 2)
        xp = np.concatenate(
            [x[b, h * HQ : (h + 1) * HQ], x[b, (1 - h) * HQ : (2 - h) * HQ]], axis=0
        )
        x8, r8 = _split8(xp)                    # [k, d]
        x8t = np.ascontiguousarray(x8.astype(np.float32).T).astype(ml_dtypes.float8_e4m3)
        r8t = np.ascontiguousarray(r8.astype(np.float32).T).astype(ml_dtypes.float8_e4m3)
        in_maps.append(
            {
                "m8": m8p, "mr8": mr8p,
                "x8t": _pair_t(x8t),
                "r8tq": _pair_t(r8t[:, 0:HQ]),
                "x8n": _pair_t(x8), "r8n": _pair_t(r8),
                "wv8": wv8p, "wvr8": wvr8p,
            }
        )
    res = None
    for attempt in range(3):
        try:
            res = run_bass_kernel_spmd(nc, in_maps, list(range(N_CORES)))
            break
        except Exception:
            # The axon PJRT path sporadically fails with
            # NRT_EXEC_UNIT_UNRECOVERABLE on an otherwise-good program;
            # a retry on a fresh execute has always succeeded.
            if attempt == 2:
                raise
    assert res is not None
    outp = np.empty((B, S, O), dtype=np.float32)
    for c in range(N_CORES):
        b, h = divmod(c, 2)
        outp[b, h * HQ : (h + 1) * HQ] = res.results[c]["outT"].astype(np.float32).T
    return outp
